# revision 1
# baseline (speedup 1.0000x reference)
"""Trainium2 Bass kernel for nn_Attention_41686952575399 (sparse attention).

Sharding: data-parallel over batch (2 groups of 4 cores) x tensor-parallel over
heads (4 heads per core). Device-side AllGather of combined heads within each
batch group; each core then computes a 256-wide dout slice of the output
projection for all tokens of its batch element.

Dataflow is fully transposed (features on SBUF partitions, tokens on the free
axis), so attention probabilities come out of the tensor engine already in the
layout the P@V matmul needs and no per-tile transposes are required. Softmax
is computed without max-subtraction (scores*scale is bounded by ~3.2 for this
model's initialization scale) with the denominator fused into the V matmul via
an appended ones-column. All per-head tensors live on partitions 0..63 so
every vector/scalar op is partition-aligned.
"""
import os
import sys

sys.path.insert(0, "/opt/trn_rl_repo")

DEBUG = os.environ.get("BASSK_DEBUG") == "1"

import numpy as np

from concourse import bacc, bass, mybir, tile
from concourse.bass_utils import run_bass_kernel_spmd

B, N, DIM = 2, 1024, 1024
H, DH = 16, 64
WIN, CB = 64, 16
NB = N // CB               # 64 compressed blocks
HPC = 4                    # heads per core
NCORES = 8
GROUPS = [[0, 1, 2, 3], [4, 5, 6, 7]]
F32 = mybir.dt.float32
MM_DT = mybir.dt.float32r  # fast full-precision-ish PE mode
NEG = -1e30
EPS = float(np.finfo(np.float32).eps)
SCALE = float(DH ** -0.5)
NF = 3 * HPC * DH + 3      # 771 projection output features (q,k,v slices + Ws)
KT = NB + 1                # 65: conv block columns + pos-embedding column

AL = mybir.AluOpType
AF = mybir.ActivationFunctionType


def _r(ap):
    """Bitcast a fp32 AP to the matmul dtype (float32r runs the PE at full
    rate for moving dims >= 256)."""
    return ap.bitcast(MM_DT)


def build_program() -> bass.Bass:
    nc = bacc.Bacc("TRN2", target_bir_lowering=False, debug=False,
                   num_devices=NCORES)

    inpT_d = nc.dram_tensor("inpT", [DIM, N], F32, kind="ExternalInput")
    wall_d = nc.dram_tensor("w_all", [DIM, NF], F32, kind="ExternalInput")
    cwk_d = nc.dram_tensor("cw_k", [DH, HPC, CB, DH], F32, kind="ExternalInput")
    cwv_d = nc.dram_tensor("cw_v", [DH, HPC, CB, DH], F32, kind="ExternalInput")
    posk_d = nc.dram_tensor("pos_k", [DH, HPC, CB], F32, kind="ExternalInput")
    posv_d = nc.dram_tensor("pos_v", [DH, HPC, CB], F32, kind="ExternalInput")
    kcb_d = nc.dram_tensor("kcb", [DH, HPC], F32, kind="ExternalInput")
    vcb_d = nc.dram_tensor("vcb", [DH, HPC], F32, kind="ExternalInput")
    bs_d = nc.dram_tensor("bs_t", [3, 1], F32, kind="ExternalInput")
    rms_d = nc.dram_tensor("rms_t", [128, 8], F32, kind="ExternalInput")
    wout_d = nc.dram_tensor("woutS", [128, 8, 256], F32, kind="ExternalInput")
    ones_d = nc.dram_tensor("ones_c", [128, 8], F32, kind="ExternalInput")
    ident_d = nc.dram_tensor("ident_c", [128, 128], F32, kind="ExternalInput")
    outT_d = nc.dram_tensor("outT", [256, N], F32, kind="ExternalOutput")
    dbg = {}
    if DEBUG:
        dbg["s"] = nc.dram_tensor("dbg_s", [1, N], F32, kind="ExternalOutput")
        dbg["w3"] = nc.dram_tensor("dbg_w3", [3, N], F32, kind="ExternalOutput")
        dbg["qkvT"] = nc.dram_tensor("dbg_qkvT", [DH, 12, N + 2 * CB], F32,
                                     kind="ExternalOutput")
        dbg["kbT"] = nc.dram_tensor("dbg_kbT", [DH, CB, KT + 1], F32,
                                    kind="ExternalOutput")
        dbg["ck_f"] = nc.dram_tensor("dbg_ck_f", [DH, NB], F32, kind="ExternalOutput")
        dbg["cv_aug"] = nc.dram_tensor("dbg_cv_aug", [NB, DH + 1], F32,
                                       kind="ExternalOutput")
        dbg["pc"] = nc.dram_tensor("dbg_pc", [NB, N], F32, kind="ExternalOutput")
        dbg["pw"] = nc.dram_tensor("dbg_pw", [128, 8, 256], F32, kind="ExternalOutput")
        dbg["vnat"] = nc.dram_tensor("dbg_vnat", [128, 8, DH + 1], F32,
                                     kind="ExternalOutput")
        dbg["oc"] = nc.dram_tensor("dbg_oc", [DH + 1, N], F32, kind="ExternalOutput")
        dbg["ow"] = nc.dram_tensor("dbg_ow", [DH + 1, N], F32, kind="ExternalOutput")
        dbg["comb"] = nc.dram_tensor("dbg_comb", [DH, HPC, N], F32,
                                     kind="ExternalOutput")
        dbg["cmb"] = nc.dram_tensor("dbg_cmb", [128, 8, N], F32,
                                    kind="ExternalOutput")

    with tile.TileContext(nc) as tc:
        _body(nc, tc, inpT_d, wall_d, cwk_d, cwv_d, posk_d, posv_d,
              kcb_d, vcb_d, bs_d, rms_d, wout_d, outT_d, ones_d, ident_d, dbg)
    nc.compile()
    return nc


def _body(nc, tc, inpT_d, wall_d, cwk_d, cwv_d, posk_d, posv_d,
          kcb_d, vcb_d, bs_d, rms_d, wout_d, outT_d, ones_d, ident_d, dbg):
    mm = nc.tensor.matmul

    # ----- long-lived constants -----------------------------------------
    const_cm = tc.tile_pool(name="const", bufs=1)
    const = const_cm.__enter__()
    ones_col = const.tile([128, 1], F32, name="ones_col")
    ident = const.tile([128, 128], F32, name="ident")
    cmask = const.tile([64, N], F32, name="cmask")
    wmask = const.tile([128, 256], F32, name="wmask")
    rms_sb = const.tile([128, 8], F32, name="rms_sb")
    bs_sb = const.tile([3, 1], F32, name="bs_sb")
    kcb_sb = const.tile([DH, HPC], F32, name="kcb_sb")
    vcb_sb = const.tile([DH, HPC], F32, name="vcb_sb")
    s_row = const.tile([1, N], F32, name="s_row")
    s_tmp = const.tile([1, N], F32, name="s_tmp")
    eps_sb = const.tile([1, 1], F32, name="eps_sb")
    s_bcast = const.tile([128, N], F32, name="s_bcast")
    w3r = const.tile([3, N], F32, name="w3r")
    w3_sb = const.tile([3, N], F32, name="w3_sb")
    w1_row = const.tile([1, N], F32, name="w1_row")
    wout_sb = const.tile([128, 8, 256], F32, name="wout_sb")
    combT = const.tile([DH, HPC, N], F32, name="combT")

    nc.gpsimd.dma_start(out=_r(ones_col[:]), in_=ones_d.ap()[:, 0:1])
    nc.gpsimd.memset(eps_sb[:], EPS)
    nc.gpsimd.dma_start(out=_r(ident[:]), in_=ident_d.ap())
    # compressed-block causal mask: block c visible to token t iff t >= 16c+15
    nc.gpsimd.memset(cmask[:], 0.0)
    nc.gpsimd.affine_select(out=cmask[:], in_=cmask[:], compare_op=AL.is_ge,
                            fill=NEG, base=-15, channel_multiplier=-16,
                            pattern=[[1, N]])
    # window mask on a [key r, query j] tile: visible iff r <= j <= r+63
    nc.gpsimd.memset(wmask[:], 0.0)
    nc.gpsimd.affine_select(out=wmask[:], in_=wmask[:], compare_op=AL.is_ge,
                            fill=NEG, base=0, channel_multiplier=-1,
                            pattern=[[1, 256]])
    nc.gpsimd.affine_select(out=wmask[:], in_=wmask[:], compare_op=AL.is_ge,
                            fill=NEG, base=63, channel_multiplier=1,
                            pattern=[[-1, 256]])

    nc.sync.dma_start(out=rms_sb[:], in_=rms_d.ap())
    nc.sync.dma_start(out=bs_sb[:], in_=bs_d.ap())
    nc.sync.dma_start(out=kcb_sb[:], in_=kcb_d.ap())
    nc.sync.dma_start(out=vcb_sb[:], in_=vcb_d.ap())
    nc.gpsimd.dma_start(out=_r(wout_sb[:]), in_=wout_d.ap())

    # ----- stage 1+2: RMS stats + fused qkv/Ws projection ---------------
    # qkvT column j: 4*part + head (part 0=q, 1=k, 2=v), cols N..N+15 hold
    # the intra-block positional embeddings for the conv's extra column.
    qkvT, qkvT_free = tc.tile([DH, 3 * HPC, N + 2 * CB], F32, name="qkvT")
    x_sb, x_free = tc.tile([128, 8, N], F32, name="x_sb")
    w_sb, w_free = tc.tile([128, 8, NF], F32, name="w_sb")

    for k in range(8):
        nc.gpsimd.dma_start(out=_r(x_sb[:, k, :]), in_=inpT_d.ap()[128 * k:128 * (k + 1), :])
        nc.gpsimd.dma_start(out=_r(w_sb[:, k, :]), in_=wall_d.ap()[128 * k:128 * (k + 1), :])
    nc.gpsimd.dma_start(out=_r(qkvT[:, 4:8, N:N + CB]), in_=posk_d.ap())
    nc.gpsimd.dma_start(out=_r(qkvT[:, 8:12, N:N + CB]), in_=posv_d.ap())
    # fp32r matmuls need an even moving dim: pad the conv with a 66th
    # (zero) block column
    nc.gpsimd.memset(qkvT[:, 4:12, N + CB:N + 2 * CB], 0.0)

    psP_cm = tc.tile_pool(name="psP", bufs=4, space="PSUM")
    psP = psP_cm.__enter__()
    sqp_cm = tc.tile_pool(name="sqp", bufs=2)
    sqp = sqp_cm.__enter__()

    # sum of squares over dim via ones-matmul on squared tiles
    ps_s = [psP.tile([1, 512], F32, name=f"ps_s{ch}", bufs=1) for ch in range(2)]
    for k in range(8):
        sq = sqp.tile([128, N], F32, name="sq")
        if k % 2 == 0:
            nc.scalar.activation(_r(sq[:]), x_sb[:, k, :], AF.Square)
        else:
            nc.vector.tensor_tensor(_r(sq[:]), x_sb[:, k, :], x_sb[:, k, :], op=AL.mult)
        for ch in range(2):
            mm(ps_s[ch][:], _r(ones_col[:]), _r(sq[:, 512 * ch:512 * (ch + 1)]),
               start=(k == 0), stop=(k == 7))
    for ch in range(2):
        nc.scalar.activation(s_tmp[0:1, 512 * ch:512 * (ch + 1)], ps_s[ch][:],
                             AF.Sqrt, bias=eps_sb[:], scale=1.0 / DIM)
    nc.vector.reciprocal(s_row[:], s_tmp[:])
    nc.gpsimd.partition_broadcast(s_bcast[:], s_row[:])

    # fold rms_w into the projection weights (per-partition scalar)
    for k in range(8):
        nc.vector.tensor_scalar(out=_r(w_sb[:, k, :]), in0=w_sb[:, k, :],
                                scalar1=rms_sb[:, k:k + 1], scalar2=None,
                                op0=AL.mult)

    # qkvT[:, j, t] = (W_eff.T @ inpT)[feat, t] * s[t]; psum rows 64..127
    # belong to the odd head of the feature tile and are moved down to
    # partitions 0..63 via a partition-shifting SBUF->SBUF DMA.
    for f in range(7):
        for ch in range(2):
            pp = psP.tile([128, 512], F32, name="pp")
            sl = slice(512 * ch, 512 * (ch + 1))
            M = 128 if f < 6 else 3
            for k in range(8):
                mm(pp[:M, :], _r(w_sb[:, k, 128 * f:128 * f + M]),
                   _r(x_sb[:, k, sl]), start=(k == 0), stop=(k == 7))
            if f < 6:
                jA = 4 * (f // 2) + 2 * (f % 2)
                nc.vector.tensor_tensor(_r(qkvT[:, jA, sl]), pp[0:64, :],
                                        s_bcast[0:64, sl], op=AL.mult)
                stage = sqp.tile([128, 512], F32, name="stage")
                nc.vector.tensor_tensor(_r(stage[64:128, :]), pp[64:128, :],
                                        s_bcast[64:128, sl], op=AL.mult)
                nc.sync.dma_start(out=_r(qkvT[:, jA + 1, sl]),
                                  in_=_r(stage[64:128, :]))
            else:
                nc.vector.tensor_tensor(w3r[:, sl], pp[:3, :],
                                        s_bcast[:3, sl], op=AL.mult)
    nc.scalar.activation(w3_sb[:], w3r[:], AF.Sigmoid, bias=bs_sb[:])
    if DEBUG:
        nc.sync.dma_start(out=dbg["s"].ap(), in_=s_row[:])
        nc.sync.dma_start(out=dbg["w3"].ap(), in_=w3_sb[:])
    nc.sync.dma_start(out=w1_row[:], in_=w3_sb[1:2, :])

    sqp_cm.__exit__(None, None, None)
    psP_cm.__exit__(None, None, None)
    w_free()
    x_free()

    # ----- stage 3-6: per-head attention --------------------------------
    cwp_cm = tc.tile_pool(name="cwp", bufs=1)
    cwp = cwp_cm.__enter__()
    cwk_sb = cwp.tile([DH, HPC, CB, DH], F32, name="cwk_sb")
    cwv_sb = cwp.tile([DH, HPC, CB, DH], F32, name="cwv_sb")
    nc.gpsimd.dma_start(out=_r(cwk_sb[:]), in_=cwk_d.ap())
    nc.gpsimd.dma_start(out=_r(cwv_sb[:]), in_=cwv_d.ap())

    psA_cm = tc.tile_pool(name="psA", bufs=3, space="PSUM")
    psA = psA_cm.__enter__()
    psO_cm = tc.tile_pool(name="psO", bufs=1, space="PSUM")
    psO = psO_cm.__enter__()
    pat_cm = tc.tile_pool(name="attn", bufs=1)
    pat = pat_cm.__enter__()
    pat2_cm = tc.tile_pool(name="attn2", bufs=2)
    pat2 = pat2_cm.__enter__()

    for h in range(HPC):
        qT = qkvT[:, h, 0:N]
        kTp = qkvT[:, 4 + h, :].rearrange("p (c t) -> p t c", t=CB)
        vTp = qkvT[:, 8 + h, :].rearrange("p (c t) -> p t c", t=CB)
        kT = qkvT[:, 4 + h, 0:N]
        vT = qkvT[:, 8 + h, 0:N]

        # -- compression conv: ckT[o,c] / cv[c,o]; c=NB is the pos column --
        # de-interleave tokens-within-block to the middle axis so each
        # per-t matmul reads a contiguous [64, 65] slab
        kbT = pat2.tile([DH, CB, KT + 1], F32, name="kbT", bufs=1)
        nc.vector.tensor_copy(_r(kbT[:]), kTp)
        vbT = pat2.tile([DH, CB, KT + 1], F32, name="vbT", bufs=1)
        nc.scalar.copy(_r(vbT[:]), vTp)

        ps_ck = psA.tile([DH, KT + 1], F32, name="ps_ck", tag="psa")
        for t in range(CB):
            mm(ps_ck[:], _r(cwk_sb[:, h, t, :]), _r(kbT[:, t, :]),
               start=(t == 0), stop=(t == CB - 1))
        ck_sb = pat2.tile([DH, KT + 1], F32, name="ck_sb", bufs=1)
        nc.scalar.copy(ck_sb[:], ps_ck[:])
        ck_f = pat2.tile([DH, NB], F32, name="ck_f")
        nc.vector.tensor_scalar(out=_r(ck_f[:]), in0=ck_sb[:, 0:NB],
                                scalar1=ck_sb[:, NB:NB + 1],
                                scalar2=kcb_sb[:, h:h + 1],
                                op0=AL.add, op1=AL.add)

        ps_cv = psA.tile([DH, KT + 1], F32, name="ps_cv", tag="psa")
        for t in range(CB):
            mm(ps_cv[:], _r(cwv_sb[:, h, t, :]), _r(vbT[:, t, :]),
               start=(t == 0), stop=(t == CB - 1))
        cv_sb = pat2.tile([DH, KT + 1], F32, name="cv_sb", bufs=1)
        nc.scalar.copy(cv_sb[:], ps_cv[:])
        cvT_f = pat2.tile([DH, NB], F32, name="cvT_f")
        nc.vector.tensor_scalar(out=_r(cvT_f[:]), in0=cv_sb[:, 0:NB],
                                scalar1=cv_sb[:, NB:NB + 1],
                                scalar2=vcb_sb[:, h:h + 1],
                                op0=AL.add, op1=AL.add)
        # natural [block, dh] orientation with a leading ones column so the
        # AV matmul emits the softmax denominator on partition 0
        ps_cvt = psA.tile([NB, DH], F32, name="ps_cvt", tag="psa")
        nc.tensor.transpose(_r(ps_cvt[:]), _r(cvT_f[:]), _r(ident[0:64, 0:64]))
        cv_aug = pat2.tile([NB, DH + 1], F32, name="cv_aug")
        nc.scalar.copy(_r(cv_aug[:, 0:DH]), ps_cvt[:])
        nc.gpsimd.dma_start(out=_r(cv_aug[:, DH:DH + 1]),
                            in_=ones_d.ap()[0:64, 0:1])

        # -- compressed branch: ScT [c,t] -> exp -> (cv_aug).T @ P --------
        pc = pat.tile([NB, N], F32, name="pc")
        ps_oc = [psO.tile([DH + 1, 512], F32, name=f"ps_oc{ch}") for ch in range(2)]
        for ch in range(2):
            sl = slice(512 * ch, 512 * (ch + 1))
            ps_sc = psA.tile([NB, 512], F32, name="ps_sc", tag="psa")
            mm(ps_sc[:], _r(ck_f[:]), _r(qT[:, sl]), start=True, stop=True)
            nc.vector.tensor_tensor(ps_sc[:], ps_sc[:], cmask[:, sl], op=AL.add)
            nc.scalar.activation(_r(pc[:, sl]), ps_sc[:], AF.Exp, scale=SCALE)
            mm(ps_oc[ch][:], _r(cv_aug[:]), _r(pc[:, sl]), start=True, stop=True)

        # -- sliding window branch: SwT [key r, query j] per key tile -----
        pw = pat.tile([128, 8, 256], F32, name="pw")
        for kt in range(8):
            nq = 256 if kt < 7 else 128
            ps_sw = psA.tile([128, 256], F32, name="ps_sw", tag="psa")
            mm(ps_sw[:, :nq], _r(kT[:, 128 * kt:128 * (kt + 1)]),
               _r(qT[:, 128 * kt:128 * kt + nq]), start=True, stop=True)
            nc.vector.tensor_tensor(ps_sw[:, :nq], ps_sw[:, :nq], wmask[:, :nq],
                                    op=AL.add)
            nc.scalar.activation(_r(pw[:, kt, :nq]), ps_sw[:, :nq], AF.Exp,
                                 scale=SCALE)

        # v in natural [token, dh] layout + ones column (via PE transpose)
        vnat = pat.tile([128, 8, DH + 1], F32, name="vnat")
        for g in range(8):
            ps_vt = psA.tile([128, DH], F32, name="ps_vt", tag="psa")
            nc.tensor.transpose(_r(ps_vt[:]), _r(vT[:, 128 * g:128 * (g + 1)]),
                                _r(ident[0:64, 0:64]))
            nc.scalar.copy(_r(vnat[:, g, 0:DH]), ps_vt[:])
        nc.gpsimd.dma_start(out=_r(vnat[:, :, DH:DH + 1]),
                            in_=ones_d.ap()[:, 0:8])

        ps_ow = [psO.tile([DH + 1, 512], F32, name=f"ps_ow{ch}") for ch in range(2)]
        for qt in range(8):
            dst = ps_ow[qt // 4][:, (qt % 4) * 128:(qt % 4) * 128 + 128]
            if qt == 0:
                mm(dst, _r(vnat[:, 0, :]), _r(pw[:, 0, 0:128]),
                   start=True, stop=True)
            else:
                mm(dst, _r(vnat[:, qt - 1, :]), _r(pw[:, qt - 1, 128:256]),
                   start=True, stop=False)
                mm(dst, _r(vnat[:, qt, :]), _r(pw[:, qt, 0:128]),
                   start=False, stop=True)

        # -- mix the two branches with the learned gates ------------------
        # reciprocal of the fused denominators (rows at partition 64 of
        # the psum outputs), then DMA-shift the result rows to partition 0
        # (HW partition_broadcast always reads the tile's partition 0)
        sc64 = pat.tile([65, N], F32, name="sc64")
        sw64 = pat.tile([65, N], F32, name="sw64")
        for ch in range(2):
            sl = slice(512 * ch, 512 * (ch + 1))
            nc.vector.reciprocal(sc64[64:65, sl], ps_oc[ch][DH:DH + 1, :])
            nc.vector.reciprocal(sw64[64:65, sl], ps_ow[ch][DH:DH + 1, :])
        sc_row = pat.tile([1, N], F32, name="sc_row")
        sw_row = pat.tile([1, N], F32, name="sw_row")
        nc.sync.dma_start(out=sc_row[:], in_=sc64[64:65, :])
        nc.sync.dma_start(out=sw_row[:], in_=sw64[64:65, :])
        nc.vector.tensor_tensor(sc_row[:], sc_row[:], w3_sb[0:1, :], op=AL.mult)
        nc.vector.tensor_tensor(sw_row[:], sw_row[:], w1_row[:], op=AL.mult)
        # tokens 0..14 see no compressed block: den==0 -> force gate to 0
        nc.vector.memset(sc_row[0:1, 0:15], 0.0)
        sc_b = pat.tile([DH, N], F32, name="sc_b")
        sw_b = pat.tile([DH, N], F32, name="sw_b")
        nc.gpsimd.partition_broadcast(sc_b[:], sc_row[:])
        nc.gpsimd.partition_broadcast(sw_b[:], sw_row[:])
        mixt = pat.tile([DH, N], F32, name="mixt")
        for ch in range(2):
            sl = slice(512 * ch, 512 * (ch + 1))
            nc.vector.tensor_tensor(mixt[:, sl], ps_oc[ch][0:DH, :],
                                    sc_b[:, sl], op=AL.mult)
            nc.vector.tensor_tensor(combT[:, h, sl], ps_ow[ch][0:DH, :],
                                    sw_b[:, sl], op=AL.mult)
            nc.vector.tensor_tensor(combT[:, h, sl], combT[:, h, sl],
                                    mixt[:, sl], op=AL.add)
        if DEBUG and h == 0:
            nc.sync.dma_start(out=dbg["qkvT"].ap(), in_=qkvT[:])
            nc.sync.dma_start(out=dbg["kbT"].ap(), in_=kbT[:])
            nc.sync.dma_start(out=dbg["ck_f"].ap(), in_=ck_f[:])
            nc.sync.dma_start(out=dbg["cv_aug"].ap(), in_=cv_aug[:])
            nc.sync.dma_start(out=dbg["pc"].ap(), in_=pc[:])
            nc.sync.dma_start(out=dbg["pw"].ap(), in_=pw[:])
            nc.sync.dma_start(out=dbg["vnat"].ap(), in_=vnat[:])
            dbg_oc_sb = pat2.tile([DH + 1, N], F32, name="dbg_oc_sb", bufs=1)
            dbg_ow_sb = pat2.tile([DH + 1, N], F32, name="dbg_ow_sb", bufs=1)
            for ch in range(2):
                sl = slice(512 * ch, 512 * (ch + 1))
                nc.scalar.copy(dbg_oc_sb[:, sl], ps_oc[ch][:])
                nc.scalar.copy(dbg_ow_sb[:, sl], ps_ow[ch][:])
            nc.sync.dma_start(out=dbg["oc"].ap(), in_=dbg_oc_sb[:])
            nc.sync.dma_start(out=dbg["ow"].ap(), in_=dbg_ow_sb[:])

    pat2_cm.__exit__(None, None, None)
    pat_cm.__exit__(None, None, None)
    psO_cm.__exit__(None, None, None)
    psA_cm.__exit__(None, None, None)
    cwp_cm.__exit__(None, None, None)
    qkvT_free()

    # ----- stage 7: AllGather heads within batch group + output proj ----
    dram_cm = tc.tile_pool(name="dram", bufs=1, space="DRAM")
    dram = dram_cm.__enter__()
    cc_in = dram.tile([HPC * DH, N], F32, name="cc_in")
    cc_out = dram.tile([4 * HPC * DH, N], F32, name="cc_out")

    if DEBUG:
        nc.sync.dma_start(out=dbg["comb"].ap(), in_=combT[:])
    nc.sync.dma_start(out=cc_in[:].rearrange("(hh p) n -> p hh n", p=64),
                      in_=combT[:])
    nc.gpsimd.collective_compute(
        "AllGather", AL.bypass, replica_groups=GROUPS,
        ins=[cc_in[:].opt()], outs=[cc_out[:].opt()])

    cmb_sb, cmb_free = tc.tile([128, 8, N], F32, name="cmb_sb")
    outT_sb, outT_sb_free = tc.tile([128, 2, N], F32, name="outT_sb")
    for k in range(8):
        nc.gpsimd.dma_start(out=_r(cmb_sb[:, k, :]),
                          in_=cc_out[128 * k:128 * (k + 1), :])

    if DEBUG:
        nc.sync.dma_start(out=dbg["cmb"].ap(), in_=cmb_sb[:])
    psW_cm = tc.tile_pool(name="psW", bufs=4, space="PSUM")
    psW = psW_cm.__enter__()
    for m in range(2):
        for ch in range(2):
            sl = slice(512 * ch, 512 * (ch + 1))
            po = psW.tile([128, 512], F32, name="po")
            for k in range(8):
                mm(po[:], _r(wout_sb[:, k, 128 * m:128 * (m + 1)]),
                   _r(cmb_sb[:, k, sl]), start=(k == 0), stop=(k == 7))
            nc.scalar.copy(outT_sb[:, m, sl], po[:])
    nc.sync.dma_start(out=outT_d.ap().rearrange("(m p) n -> p m n", p=128),
                      in_=outT_sb[:])

    psW_cm.__exit__(None, None, None)
    outT_sb_free()
    cmb_free()
    dram_cm.__exit__(None, None, None)
    const_cm.__exit__(None, None, None)


# --------------------------------------------------------------------------
_CACHE: dict = {}


def _get_nc() -> bass.Bass:
    if "nc" not in _CACHE:
        _CACHE["nc"] = build_program()
    return _CACHE["nc"]


def _prep_core(c: int, inputs: dict) -> dict:
    b, r = c // 4, c % 4
    hs = HPC * r
    f32 = np.float32
    inp = np.asarray(inputs["inp"], f32)
    rms_w = np.asarray(inputs["rms_w"], f32)
    Wqkv = np.asarray(inputs["Wqkv"], f32)
    k_pos = np.asarray(inputs["k_pos"], f32)
    v_pos = np.asarray(inputs["v_pos"], f32)
    k_cw = np.asarray(inputs["k_cw"], f32)
    k_cb = np.asarray(inputs["k_cb"], f32)
    v_cw = np.asarray(inputs["v_cw"], f32)
    v_cb = np.asarray(inputs["v_cb"], f32)
    Ws = np.asarray(inputs["Ws"], f32)
    bs = np.asarray(inputs["bs"], f32)
    Wout = np.asarray(inputs["Wout"], f32)

    cols = [Wqkv[:, p * H * DH + hs * DH: p * H * DH + (hs + HPC) * DH]
            for p in range(3)]
    w_all = np.ascontiguousarray(np.concatenate(cols + [Ws], axis=1))

    return {
        "inpT": np.ascontiguousarray(inp[b].T),
        "w_all": w_all,
        # [i, h, t, o] = cw[hs+h, o, i, t]
        "cw_k": np.ascontiguousarray(k_cw[hs:hs + HPC].transpose(2, 0, 3, 1)),
        "cw_v": np.ascontiguousarray(v_cw[hs:hs + HPC].transpose(2, 0, 3, 1)),
        # [i, h, t] = pos[hs+h, t, i]
        "pos_k": np.ascontiguousarray(k_pos[hs:hs + HPC].transpose(2, 0, 1)),
        "pos_v": np.ascontiguousarray(v_pos[hs:hs + HPC].transpose(2, 0, 1)),
        "kcb": np.ascontiguousarray(k_cb[hs:hs + HPC].T),
        "vcb": np.ascontiguousarray(v_cb[hs:hs + HPC].T),
        "bs_t": np.ascontiguousarray(bs[:, None]),
        "rms_t": np.ascontiguousarray(rms_w.reshape(8, 128).T),
        "woutS": np.ascontiguousarray(
            Wout[:, 256 * r:256 * (r + 1)].reshape(8, 128, 256).transpose(1, 0, 2)),
        "ones_c": np.ones((128, 8), f32),
        "ident_c": np.eye(128, dtype=f32),
    }


def kernel(**inputs) -> np.ndarray:
    nc = _get_nc()
    in_maps = [_prep_core(c, inputs) for c in range(NCORES)]
    res = run_bass_kernel_spmd(nc, in_maps, list(range(NCORES)))
    out = np.zeros((B, N, DIM), np.float32)
    for c in range(NCORES):
        b, r = c // 4, c % 4
        out[b, :, 256 * r:256 * (r + 1)] = res.results[c]["outT"].T
    return out



# revision 17
# speedup vs baseline: 1.5450x; 1.5450x over previous
"""Trainium2 Bass kernel for nn_Attention_41686952575399 (sparse attention).

Sharding: data-parallel over batch (2 groups of 4 cores) x tensor-parallel over
heads (4 heads per core). Device-side AllGather of combined heads within each
batch group; each core then computes a 256-wide dout slice of the output
projection for all tokens of its batch element.

Dataflow is fully transposed (features on SBUF partitions, tokens on the free
axis), so attention probabilities come out of the tensor engine already in the
layout the P@V matmul needs and no per-tile transposes are required. Softmax
is computed without max-subtraction (scores*scale is bounded by ~3.2 for this
model's initialization scale) with the denominator fused into the V matmul via
an appended ones-column. All per-head tensors live on partitions 0..63 so
every vector/scalar op is partition-aligned.
"""
import os
import sys

sys.path.insert(0, "/opt/trn_rl_repo")

DEBUG = os.environ.get("BASSK_DEBUG") == "1"

import numpy as np

from concourse import bacc, bass, mybir, tile
from concourse.bass_utils import run_bass_kernel_spmd

B, N, DIM = 2, 1024, 1024
H, DH = 16, 64
WIN, CB = 64, 16
NB = N // CB               # 64 compressed blocks
HPC = 4                    # heads per core
NCORES = 8
F32 = mybir.dt.float32
MM_DT = mybir.dt.float32r  # fast full-precision-ish PE mode
NEG = -1e30
EPS = float(np.finfo(np.float32).eps)
SCALE = float(DH ** -0.5)
NF = 3 * HPC * DH + 3      # 771 projection output features (q,k,v slices + Ws)
KT = NB + 1                # 65: conv block columns + pos-embedding column

AL = mybir.AluOpType
AF = mybir.ActivationFunctionType


def _r(ap):
    """Bitcast a fp32 AP to the matmul dtype (float32r runs the PE at full
    rate for moving dims >= 256)."""
    return ap.bitcast(MM_DT)


def build_program() -> bass.Bass:
    nc = bacc.Bacc("TRN2", target_bir_lowering=False, debug=False,
                   num_devices=NCORES)

    inpT_d = nc.dram_tensor("inpT", [DIM, N], F32, kind="ExternalInput")
    wall_d = nc.dram_tensor("w_all", [DIM, NF], F32, kind="ExternalInput")
    cwk_d = nc.dram_tensor("cw_k", [DH, HPC, CB, DH], F32, kind="ExternalInput")
    cwv_d = nc.dram_tensor("cw_v", [DH, HPC, CB, DH], F32, kind="ExternalInput")
    posk_d = nc.dram_tensor("pos_k", [DH, HPC, CB], F32, kind="ExternalInput")
    posv_d = nc.dram_tensor("pos_v", [DH, HPC, CB], F32, kind="ExternalInput")
    kcb_d = nc.dram_tensor("kcb", [DH, HPC], F32, kind="ExternalInput")
    vcb_d = nc.dram_tensor("vcb", [DH, HPC], F32, kind="ExternalInput")
    bs_d = nc.dram_tensor("bs_t", [3, 1], F32, kind="ExternalInput")
    rms_d = nc.dram_tensor("rms_t", [128, 8], F32, kind="ExternalInput")
    wout_d = nc.dram_tensor("woutP", [128, 2, 8, 128], F32, kind="ExternalInput")
    ones_d = nc.dram_tensor("ones_c", [128, 8], F32, kind="ExternalInput")
    ident_d = nc.dram_tensor("ident_c", [128, 128], F32, kind="ExternalInput")
    outT_d = nc.dram_tensor("outT", [DIM, N], F32, kind="ExternalOutput")
    dbg = {}
    if DEBUG:
        dbg["s"] = nc.dram_tensor("dbg_s", [1, N], F32, kind="ExternalOutput")
        dbg["w3"] = nc.dram_tensor("dbg_w3", [3, N], F32, kind="ExternalOutput")
        dbg["qkvT"] = nc.dram_tensor("dbg_qkvT", [DH, 12, N + 2 * CB], F32,
                                     kind="ExternalOutput")
        dbg["kbT"] = nc.dram_tensor("dbg_kbT", [DH, CB, KT + 1], F32,
                                    kind="ExternalOutput")
        dbg["ck_f"] = nc.dram_tensor("dbg_ck_f", [DH, NB], F32, kind="ExternalOutput")
        dbg["cv_aug"] = nc.dram_tensor("dbg_cv_aug", [NB, DH + 1], F32,
                                       kind="ExternalOutput")
        dbg["pc"] = nc.dram_tensor("dbg_pc", [NB, N], F32, kind="ExternalOutput")
        dbg["pw"] = nc.dram_tensor("dbg_pw", [128, 8, 256], F32, kind="ExternalOutput")
        dbg["vnat"] = nc.dram_tensor("dbg_vnat", [128, 8, DH + 1], F32,
                                     kind="ExternalOutput")
        dbg["oc"] = nc.dram_tensor("dbg_oc", [DH + 1, N], F32, kind="ExternalOutput")
        dbg["ow"] = nc.dram_tensor("dbg_ow", [DH + 1, N], F32, kind="ExternalOutput")
        dbg["cmb2"] = nc.dram_tensor("dbg_cmb2", [128, 2, N], F32,
                                     kind="ExternalOutput")

    with tile.TileContext(nc) as tc:
        _body(nc, tc, inpT_d, wall_d, cwk_d, cwv_d, posk_d, posv_d,
              kcb_d, vcb_d, bs_d, rms_d, wout_d, outT_d, ones_d, ident_d, dbg)
    nc.compile()
    return nc


def _body(nc, tc, inpT_d, wall_d, cwk_d, cwv_d, posk_d, posv_d,
          kcb_d, vcb_d, bs_d, rms_d, wout_d, outT_d, ones_d, ident_d, dbg):
    mm = nc.tensor.matmul

    # ----- long-lived constants -----------------------------------------
    const_cm = tc.tile_pool(name="const", bufs=1)
    const = const_cm.__enter__()
    ones_col = const.tile([128, 1], F32, name="ones_col")
    ident = const.tile([128, 128], F32, name="ident")
    cmask = const.tile([64, N], F32, name="cmask")
    wmask = const.tile([128, 256], F32, name="wmask")
    rms_sb = const.tile([128, 8], F32, name="rms_sb")
    bs_sb = const.tile([3, 1], F32, name="bs_sb")
    kcb_sb = const.tile([DH, HPC], F32, name="kcb_sb")
    vcb_sb = const.tile([DH, HPC], F32, name="vcb_sb")
    s_row = const.tile([1, N], F32, name="s_row")
    s_tmp = const.tile([1, N], F32, name="s_tmp")
    eps_sb = const.tile([1, 1], F32, name="eps_sb")
    s_bcast = const.tile([128, N], F32, name="s_bcast")
    w3r = const.tile([3, N], F32, name="w3r")
    w3_sb = const.tile([3, N], F32, name="w3_sb")
    w1_row = const.tile([1, N], F32, name="w1_row")
    wout_sb = const.tile([128, 2, 8, 128], F32, name="wout_sb")
    comb2 = const.tile([128, 2, N], F32, name="comb2")
    combT_odd = const.tile([DH, 2, N], F32, name="combT_odd")

    nc.gpsimd.dma_start(out=_r(ones_col[:]), in_=ones_d.ap()[:, 0:1])
    nc.gpsimd.memset(eps_sb[:], EPS)
    nc.gpsimd.dma_start(out=_r(ident[:]), in_=ident_d.ap())
    # compressed-block causal mask: block c visible to token t iff t >= 16c+15
    nc.gpsimd.memset(cmask[:], 0.0)
    nc.gpsimd.affine_select(out=cmask[:], in_=cmask[:], compare_op=AL.is_ge,
                            fill=NEG, base=-15, channel_multiplier=-16,
                            pattern=[[1, N]])
    # window mask on a [key r, query j] tile: visible iff r <= j <= r+63
    nc.gpsimd.memset(wmask[:], 0.0)
    nc.gpsimd.affine_select(out=wmask[:], in_=wmask[:], compare_op=AL.is_ge,
                            fill=NEG, base=0, channel_multiplier=-1,
                            pattern=[[1, 256]])
    nc.gpsimd.affine_select(out=wmask[:], in_=wmask[:], compare_op=AL.is_ge,
                            fill=NEG, base=63, channel_multiplier=1,
                            pattern=[[-1, 256]])

    nc.sync.dma_start(out=rms_sb[:], in_=rms_d.ap())
    nc.sync.dma_start(out=bs_sb[:], in_=bs_d.ap())
    nc.sync.dma_start(out=kcb_sb[:], in_=kcb_d.ap())
    nc.sync.dma_start(out=vcb_sb[:], in_=vcb_d.ap())
    nc.gpsimd.dma_start(out=_r(wout_sb[:]), in_=wout_d.ap())

    # ----- stage 1+2: RMS stats + fused qkv/Ws projection ---------------
    # qkvT column j: 4*part + head (part 0=q, 1=k, 2=v), cols N..N+15 hold
    # the intra-block positional embeddings for the conv's extra column.
    qkvT, qkvT_free = tc.tile([DH, 3 * HPC, N + 2 * CB], F32, name="qkvT")
    x_sb, x_free = tc.tile([128, 8, N], F32, name="x_sb")
    w_sb, w_free = tc.tile([128, 8, NF], F32, name="w_sb")

    for k in range(8):
        nc.gpsimd.dma_start(out=_r(x_sb[:, k, :]), in_=inpT_d.ap()[128 * k:128 * (k + 1), :])
        nc.gpsimd.dma_start(out=_r(w_sb[:, k, :]), in_=wall_d.ap()[128 * k:128 * (k + 1), :])
    nc.gpsimd.dma_start(out=_r(qkvT[:, 4:8, N:N + CB]), in_=posk_d.ap())
    nc.gpsimd.dma_start(out=_r(qkvT[:, 8:12, N:N + CB]), in_=posv_d.ap())
    # fp32r matmuls need an even moving dim: pad the conv with a 66th
    # (zero) block column
    nc.gpsimd.memset(qkvT[:, 4:12, N + CB:N + 2 * CB], 0.0)

    psP_cm = tc.tile_pool(name="psP", bufs=4, space="PSUM")
    psP = psP_cm.__enter__()
    sqp_cm = tc.tile_pool(name="sqp", bufs=2)
    sqp = sqp_cm.__enter__()

    # sum of squares over dim via ones-matmul on squared tiles
    ps_s = [psP.tile([1, 512], F32, name=f"ps_s{ch}", bufs=1) for ch in range(2)]
    for k in range(8):
        sq = sqp.tile([128, N], F32, name="sq")
        if k % 2 == 0:
            nc.scalar.activation(_r(sq[:]), x_sb[:, k, :], AF.Square)
        else:
            nc.vector.tensor_tensor(_r(sq[:]), x_sb[:, k, :], x_sb[:, k, :], op=AL.mult)
        for ch in range(2):
            mm(ps_s[ch][:], _r(ones_col[:]), _r(sq[:, 512 * ch:512 * (ch + 1)]),
               start=(k == 0), stop=(k == 7))
    for ch in range(2):
        nc.scalar.activation(s_tmp[0:1, 512 * ch:512 * (ch + 1)], ps_s[ch][:],
                             AF.Sqrt, bias=eps_sb[:], scale=1.0 / DIM)
    nc.vector.reciprocal_approx_fast(out=s_row[:], in_=s_tmp[:])
    nc.gpsimd.partition_broadcast(s_bcast[:], s_row[:])

    # fold rms_w into the projection weights (per-partition scalar)
    for k in range(8):
        nc.vector.tensor_scalar(out=_r(w_sb[:, k, :]), in0=w_sb[:, k, :],
                                scalar1=rms_sb[:, k:k + 1], scalar2=None,
                                op0=AL.mult)

    # qkvT[:, j, t] = (W_eff.T @ inpT)[feat, t] * s[t]; psum rows 64..127
    # belong to the odd head of the feature tile and are moved down to
    # partitions 0..63 via a partition-shifting SBUF->SBUF DMA.
    for f in range(7):
        for ch in range(2):
            pp = psP.tile([128, 512], F32, name="pp")
            sl = slice(512 * ch, 512 * (ch + 1))
            M = 128 if f < 6 else 3
            for k in range(8):
                mm(pp[:M, :], _r(w_sb[:, k, 128 * f:128 * f + M]),
                   _r(x_sb[:, k, sl]), start=(k == 0), stop=(k == 7))
            if f < 6:
                jA = 4 * (f // 2) + 2 * (f % 2)
                nc.vector.tensor_tensor(_r(qkvT[:, jA, sl]), pp[0:64, :],
                                        s_bcast[0:64, sl], op=AL.mult)
                stage = sqp.tile([128, 512], F32, name="stage")
                nc.vector.tensor_tensor(_r(stage[64:128, :]), pp[64:128, :],
                                        s_bcast[64:128, sl], op=AL.mult)
                nc.sync.dma_start(out=_r(qkvT[:, jA + 1, sl]),
                                  in_=_r(stage[64:128, :]))
            else:
                nc.vector.tensor_tensor(w3r[:, sl], pp[:3, :],
                                        s_bcast[:3, sl], op=AL.mult)
    nc.scalar.activation(w3_sb[:], w3r[:], AF.Sigmoid, bias=bs_sb[:])
    if DEBUG:
        nc.sync.dma_start(out=dbg["s"].ap(), in_=s_row[:])
        nc.sync.dma_start(out=dbg["w3"].ap(), in_=w3_sb[:])
    nc.sync.dma_start(out=w1_row[:], in_=w3_sb[1:2, :])

    sqp_cm.__exit__(None, None, None)
    psP_cm.__exit__(None, None, None)
    w_free()
    x_free()

    # ----- stage 3-6: per-head attention --------------------------------
    cwp_cm = tc.tile_pool(name="cwp", bufs=1)
    cwp = cwp_cm.__enter__()
    cwk_sb = cwp.tile([DH, HPC, CB, DH], F32, name="cwk_sb")
    cwv_sb = cwp.tile([DH, HPC, CB, DH], F32, name="cwv_sb")
    nc.gpsimd.dma_start(out=_r(cwk_sb[:]), in_=cwk_d.ap())
    nc.gpsimd.dma_start(out=_r(cwv_sb[:]), in_=cwv_d.ap())

    psA_cm = tc.tile_pool(name="psA", bufs=3, space="PSUM")
    psA = psA_cm.__enter__()
    psO_cm = tc.tile_pool(name="psO", bufs=1, space="PSUM")
    psO = psO_cm.__enter__()
    pat_cm = tc.tile_pool(name="attn", bufs=1)
    pat = pat_cm.__enter__()
    pat2_cm = tc.tile_pool(name="attn2", bufs=2)
    pat2 = pat2_cm.__enter__()

    for h in range(HPC):
        qT = qkvT[:, h, 0:N]
        kTp = qkvT[:, 4 + h, :].rearrange("p (c t) -> p t c", t=CB)
        vTp = qkvT[:, 8 + h, :].rearrange("p (c t) -> p t c", t=CB)
        kT = qkvT[:, 4 + h, 0:N]
        vT = qkvT[:, 8 + h, 0:N]

        # -- compression conv: ckT[o,c] / cv[c,o]; c=NB is the pos column --
        # de-interleave tokens-within-block to the middle axis so each
        # per-t matmul reads a contiguous [64, 65] slab
        kbT = pat2.tile([DH, CB, KT + 1], F32, name="kbT", bufs=1)
        nc.vector.tensor_copy(_r(kbT[:]), kTp)
        vbT = pat2.tile([DH, CB, KT + 1], F32, name="vbT", bufs=1)
        nc.scalar.copy(_r(vbT[:]), vTp)

        ps_ck = psA.tile([DH, KT + 1], F32, name="ps_ck", tag="psa")
        for t in range(CB):
            mm(ps_ck[:], _r(cwk_sb[:, h, t, :]), _r(kbT[:, t, :]),
               start=(t == 0), stop=(t == CB - 1))
        ck_sb = pat2.tile([DH, KT + 1], F32, name="ck_sb", bufs=1)
        nc.scalar.copy(ck_sb[:], ps_ck[:])
        ck_f = pat2.tile([DH, NB], F32, name="ck_f")
        nc.vector.tensor_scalar(out=_r(ck_f[:]), in0=ck_sb[:, 0:NB],
                                scalar1=ck_sb[:, NB:NB + 1],
                                scalar2=kcb_sb[:, h:h + 1],
                                op0=AL.add, op1=AL.add)

        ps_cv = psA.tile([DH, KT + 1], F32, name="ps_cv", tag="psa")
        for t in range(CB):
            mm(ps_cv[:], _r(cwv_sb[:, h, t, :]), _r(vbT[:, t, :]),
               start=(t == 0), stop=(t == CB - 1))
        cv_sb = pat2.tile([DH, KT + 1], F32, name="cv_sb", bufs=1)
        nc.scalar.copy(cv_sb[:], ps_cv[:])
        cvT_f = pat2.tile([DH, NB], F32, name="cvT_f")
        nc.vector.tensor_scalar(out=_r(cvT_f[:]), in0=cv_sb[:, 0:NB],
                                scalar1=cv_sb[:, NB:NB + 1],
                                scalar2=vcb_sb[:, h:h + 1],
                                op0=AL.add, op1=AL.add)
        # natural [block, dh] orientation with a leading ones column so the
        # AV matmul emits the softmax denominator on partition 0
        ps_cvt = psA.tile([NB, DH], F32, name="ps_cvt", tag="psa")
        nc.tensor.transpose(_r(ps_cvt[:]), _r(cvT_f[:]), _r(ident[0:64, 0:64]))
        cv_aug = pat2.tile([NB, DH + 1], F32, name="cv_aug")
        nc.scalar.copy(_r(cv_aug[:, 0:DH]), ps_cvt[:])
        nc.gpsimd.dma_start(out=_r(cv_aug[:, DH:DH + 1]),
                            in_=ones_d.ap()[0:64, 0:1])

        # -- compressed branch: ScT [c,t] -> exp -> (cv_aug).T @ P --------
        pc = pat.tile([NB, N], F32, name="pc")
        ps_oc = [psO.tile([DH + 1, 512], F32, name=f"ps_oc{ch}") for ch in range(2)]
        for ch in range(2):
            sl = slice(512 * ch, 512 * (ch + 1))
            ps_sc = psA.tile([NB, 512], F32, name="ps_sc", tag="psa")
            mm(ps_sc[:], _r(ck_f[:]), _r(qT[:, sl]), start=True, stop=True)
            nc.vector.tensor_tensor(ps_sc[:], ps_sc[:], cmask[:, sl], op=AL.add)
            nc.scalar.activation(_r(pc[:, sl]), ps_sc[:], AF.Exp, scale=SCALE)
            mm(ps_oc[ch][:], _r(cv_aug[:]), _r(pc[:, sl]), start=True, stop=True)

        # -- sliding window branch: SwT [key r, query j] per key tile -----
        pw = pat.tile([128, 8, 256], F32, name="pw")
        for kt in range(8):
            nq = 256 if kt < 7 else 128
            ps_sw = psA.tile([128, 256], F32, name="ps_sw", tag="psa")
            mm(ps_sw[:, :nq], _r(kT[:, 128 * kt:128 * (kt + 1)]),
               _r(qT[:, 128 * kt:128 * kt + nq]), start=True, stop=True)
            nc.vector.tensor_tensor(ps_sw[:, :nq], ps_sw[:, :nq], wmask[:, :nq],
                                    op=AL.add)
            nc.scalar.activation(_r(pw[:, kt, :nq]), ps_sw[:, :nq], AF.Exp,
                                 scale=SCALE)

        # v in natural [token, dh] layout + ones column (via PE transpose)
        vnat = pat.tile([128, 8, DH + 1], F32, name="vnat")
        for g in range(8):
            ps_vt = psA.tile([128, DH], F32, name="ps_vt", tag="psa")
            nc.tensor.transpose(_r(ps_vt[:]), _r(vT[:, 128 * g:128 * (g + 1)]),
                                _r(ident[0:64, 0:64]))
            nc.scalar.copy(_r(vnat[:, g, 0:DH]), ps_vt[:])
        nc.gpsimd.dma_start(out=_r(vnat[:, :, DH:DH + 1]),
                            in_=ones_d.ap()[:, 0:8])

        ps_ow = [psO.tile([DH + 1, 512], F32, name=f"ps_ow{ch}") for ch in range(2)]
        for qt in range(8):
            dst = ps_ow[qt // 4][:, (qt % 4) * 128:(qt % 4) * 128 + 128]
            if qt == 0:
                mm(dst, _r(vnat[:, 0, :]), _r(pw[:, 0, 0:128]),
                   start=True, stop=True)
            else:
                mm(dst, _r(vnat[:, qt - 1, :]), _r(pw[:, qt - 1, 128:256]),
                   start=True, stop=False)
                mm(dst, _r(vnat[:, qt, :]), _r(pw[:, qt, 0:128]),
                   start=False, stop=True)

        # -- mix the two branches with the learned gates ------------------
        # reciprocal of the fused denominators (rows at partition 64 of
        # the psum outputs), then DMA-shift the result rows to partition 0
        # (HW partition_broadcast always reads the tile's partition 0)
        sc64 = pat.tile([65, N], F32, name="sc64")
        sw64 = pat.tile([65, N], F32, name="sw64")
        for ch in range(2):
            sl = slice(512 * ch, 512 * (ch + 1))
            nc.vector.reciprocal(sc64[64:65, sl], ps_oc[ch][DH:DH + 1, :])
            nc.vector.reciprocal(sw64[64:65, sl], ps_ow[ch][DH:DH + 1, :])
        sc_row = pat.tile([1, N], F32, name="sc_row")
        sw_row = pat.tile([1, N], F32, name="sw_row")
        nc.sync.dma_start(out=sc_row[:], in_=sc64[64:65, :])
        nc.sync.dma_start(out=sw_row[:], in_=sw64[64:65, :])
        nc.vector.tensor_tensor(sc_row[:], sc_row[:], w3_sb[0:1, :], op=AL.mult)
        nc.vector.tensor_tensor(sw_row[:], sw_row[:], w1_row[:], op=AL.mult)
        # tokens 0..14 see no compressed block: den==0 -> force gate to 0
        nc.vector.memset(sc_row[0:1, 0:15], 0.0)
        sc_b = pat.tile([DH, N], F32, name="sc_b")
        sw_b = pat.tile([DH, N], F32, name="sw_b")
        nc.gpsimd.partition_broadcast(sc_b[:], sc_row[:])
        nc.gpsimd.partition_broadcast(sw_b[:], sw_row[:])
        mixt = pat.tile([DH, N], F32, name="mixt")
        dst = comb2[0:64, h // 2, :] if h % 2 == 0 else combT_odd[:, h // 2, :]
        for ch in range(2):
            sl = slice(512 * ch, 512 * (ch + 1))
            nc.vector.tensor_tensor(mixt[:, sl], ps_oc[ch][0:DH, :],
                                    sc_b[:, sl], op=AL.mult)
            nc.vector.tensor_tensor(_r(dst[:, sl]), ps_ow[ch][0:DH, :],
                                    sw_b[:, sl], op=AL.mult)
            nc.vector.tensor_tensor(_r(dst[:, sl]), dst[:, sl],
                                    mixt[:, sl], op=AL.add)
        if DEBUG and h == 0:
            nc.sync.dma_start(out=dbg["qkvT"].ap(), in_=qkvT[:])
            nc.sync.dma_start(out=dbg["kbT"].ap(), in_=kbT[:])
            nc.sync.dma_start(out=dbg["ck_f"].ap(), in_=ck_f[:])
            nc.sync.dma_start(out=dbg["cv_aug"].ap(), in_=cv_aug[:])
            nc.sync.dma_start(out=dbg["pc"].ap(), in_=pc[:])
            nc.sync.dma_start(out=dbg["pw"].ap(), in_=pw[:])
            nc.sync.dma_start(out=dbg["vnat"].ap(), in_=vnat[:])
            dbg_oc_sb = pat2.tile([DH + 1, N], F32, name="dbg_oc_sb", bufs=1)
            dbg_ow_sb = pat2.tile([DH + 1, N], F32, name="dbg_ow_sb", bufs=1)
            for ch in range(2):
                sl = slice(512 * ch, 512 * (ch + 1))
                nc.scalar.copy(dbg_oc_sb[:, sl], ps_oc[ch][:])
                nc.scalar.copy(dbg_ow_sb[:, sl], ps_ow[ch][:])
            nc.sync.dma_start(out=dbg["oc"].ap(), in_=dbg_oc_sb[:])
            nc.sync.dma_start(out=dbg["ow"].ap(), in_=dbg_ow_sb[:])

    pat2_cm.__exit__(None, None, None)
    pat_cm.__exit__(None, None, None)
    psO_cm.__exit__(None, None, None)
    psA_cm.__exit__(None, None, None)
    cwp_cm.__exit__(None, None, None)
    qkvT_free()

    # ----- stage 7: local partial output projection (host sums partials) --
    # comb2[:, s, :]: partitions 0..63 = head 2s, 64..127 = head 2s+1.
    # Odd heads were mixed into combT_odd (partitions 0..63); shift them up
    # with one partition-offsetting SBUF->SBUF DMA.
    nc.sync.dma_start(out=_r(comb2[64:128, :, :]), in_=_r(combT_odd[:]))
    if DEBUG:
        nc.sync.dma_start(out=dbg["cmb2"].ap(), in_=comb2[:])

    psW_cm = tc.tile_pool(name="psW", bufs=4, space="PSUM")
    psW = psW_cm.__enter__()
    outS, outS_free = tc.tile([128, 8, N], F32, name="outS")
    for m in range(8):
        for ch in range(2):
            sl = slice(512 * ch, 512 * (ch + 1))
            po = psW.tile([128, 512], F32, name="po")
            for s in range(2):
                mm(po[:], _r(wout_sb[:, s, m, :]), _r(comb2[:, s, sl]),
                   start=(s == 0), stop=(s == 1))
            if ch == 0:
                nc.scalar.copy(outS[:, m, sl], po[:])
            else:
                nc.vector.tensor_copy(outS[:, m, sl], po[:])
        eng = nc.sync if m % 2 == 0 else nc.gpsimd
        eng.dma_start(out=outT_d.ap()[128 * m:128 * (m + 1), :],
                      in_=outS[:, m, :])

    psW_cm.__exit__(None, None, None)
    outS_free()
    const_cm.__exit__(None, None, None)


# --------------------------------------------------------------------------
_CACHE: dict = {}


def _get_nc() -> bass.Bass:
    if "nc" not in _CACHE:
        _CACHE["nc"] = build_program()
    return _CACHE["nc"]


def _prep_core(c: int, inputs: dict) -> dict:
    b, r = c // 4, c % 4
    hs = HPC * r
    f32 = np.float32
    inp = np.asarray(inputs["inp"], f32)
    rms_w = np.asarray(inputs["rms_w"], f32)
    Wqkv = np.asarray(inputs["Wqkv"], f32)
    k_pos = np.asarray(inputs["k_pos"], f32)
    v_pos = np.asarray(inputs["v_pos"], f32)
    k_cw = np.asarray(inputs["k_cw"], f32)
    k_cb = np.asarray(inputs["k_cb"], f32)
    v_cw = np.asarray(inputs["v_cw"], f32)
    v_cb = np.asarray(inputs["v_cb"], f32)
    Ws = np.asarray(inputs["Ws"], f32)
    bs = np.asarray(inputs["bs"], f32)
    Wout = np.asarray(inputs["Wout"], f32)

    cols = [Wqkv[:, p * H * DH + hs * DH: p * H * DH + (hs + HPC) * DH]
            for p in range(3)]
    w_all = np.ascontiguousarray(np.concatenate(cols + [Ws], axis=1))

    return {
        "inpT": np.ascontiguousarray(inp[b].T),
        "w_all": w_all,
        # [i, h, t, o] = cw[hs+h, o, i, t]
        "cw_k": np.ascontiguousarray(k_cw[hs:hs + HPC].transpose(2, 0, 3, 1)),
        "cw_v": np.ascontiguousarray(v_cw[hs:hs + HPC].transpose(2, 0, 3, 1)),
        # [i, h, t] = pos[hs+h, t, i]
        "pos_k": np.ascontiguousarray(k_pos[hs:hs + HPC].transpose(2, 0, 1)),
        "pos_v": np.ascontiguousarray(v_pos[hs:hs + HPC].transpose(2, 0, 1)),
        "kcb": np.ascontiguousarray(k_cb[hs:hs + HPC].T),
        "vcb": np.ascontiguousarray(v_cb[hs:hs + HPC].T),
        "bs_t": np.ascontiguousarray(bs[:, None]),
        "rms_t": np.ascontiguousarray(rms_w.reshape(8, 128).T),
        # woutP[p, s, m, j] = Wout[256r + 128s + p, 128m + j]
        "woutP": np.ascontiguousarray(
            Wout[256 * r:256 * (r + 1), :].reshape(2, 128, 8, 128)
            .transpose(1, 0, 2, 3)),
        "ones_c": np.ones((128, 8), f32),
        "ident_c": np.eye(128, dtype=f32),
    }


def kernel(**inputs) -> np.ndarray:
    nc = _get_nc()
    in_maps = [_prep_core(c, inputs) for c in range(NCORES)]
    res = run_bass_kernel_spmd(nc, in_maps, list(range(NCORES)))
    out = np.zeros((B, N, DIM), np.float32)
    for c in range(NCORES):
        b = c // 4
        out[b] += res.results[c]["outT"].T
    return out

